# revision 14
# baseline (speedup 1.0000x reference)
"""3x3 conv (im2col formulation) on TRN2, data-parallel over batch, 8 cores.

Full inputs: x [32, 128, 56, 56] f32, w [1152, 256] f32 (row = c*9 + kh*3 + kw).
Full output: [32, 256, 56, 56] f32.

Each core processes 4 images; no collectives. Per core, per (image, oc-half,
8-row band) the 9 conv taps accumulate w_tap.T @ x_shifted into a [128, 8, 56]
f32 PSUM bank:
  - 7 taps run in fp16 (host-cast, w pre-scaled by 64 = exponent shift):
    3 full-width (dw=0, N=448) + 4 width-55 (dh=+-1, dw=+-1) matmuls.
  - The 2 (dh=0, dw=+-1) taps are fused into ONE fp8e4 DoubleRow matmul
    (2 weights/PE cell, 2 MACs/cycle): host bakes the +-1 column shifts
    (with edge zeros) into a [C, 2, H*W] plane tensor, so the pair is a
    full-width N=448 matmul streaming 896 fp8 values. Band cost drops from
    3984 to ~3610 PE cycles (-9.4%). Measured end-to-end rel err ~1.95e-2
    (gate 2e-2; deterministic: host-side round-to-nearest quantization,
    fixed PSUM accumulation order).
  - Output staged as fp16 carrying 64x values (PSUM copy converts); host
    divides by 64 after gather (exact, exponent-only).
Lead-in: first data chunks are split across both HWDGE rings (w16/w8 halves
on sync, x/y heads on scalar) so the first real matmul starts ~10us; 8 bf16
warmup matmuls keep the PE HAM activity window busy from ~6.7us so the real
stream runs at 2.4GHz. Tail: the last band is computed as 6-row + 2-row PSUM
groups so the final trailing DMA is small and the big chunk's HBM receipt
overlaps the 2-row group's matmuls.
"""

import numpy as np
import ml_dtypes

import concourse.bass as bass  # noqa: F401  (registers AP types)
import concourse.mybir as mybir
import concourse.tile as tile
from concourse import bacc, bass_utils

B, C, H, W = 32, 128, 56, 56
COUT = 256
NCORES = 8
BPC = B // NCORES  # images per core
HP = H + 2
HW = H * W
F32 = mybir.dt.float32
BF16 = mybir.dt.bfloat16
F16 = mybir.dt.float16
F8 = mybir.dt.float8e4
F16_NP = np.float16
F8_NP = ml_dtypes.float8_e4m3

# fp16 taps: full-width dw=0 first (N=448), then the four corner taps (N=440).
TAPS16 = [(-1, 0), (0, 0), (1, 0), (-1, -1), (-1, 1), (1, -1), (1, 1)]
PAIR = [(0, -1), (0, 1)]  # fused fp8 DoubleRow pair
WSCALE = 64.0  # pure exponent shift; undone on host after gather
HROWS = 8
HT = H // HROWS  # bands per image
NWARM = 8

_cached_nc = None


def _band(nc, pt, xp, ypl, wbuf16, wbuf8, oc, r0, rows):
    """Emit the 8 matmuls of one PSUM band covering output rows r0..r0+rows."""
    n16 = rows * W
    for t, (dh, dw) in enumerate(TAPS16):
        h0 = r0 + dh + 1
        if dw == 0:
            rhs = xp[:, h0 : h0 + rows, :]
            dst = pt[:]
        elif dw == -1:
            rhs = xp[:, h0 : h0 + rows, 0 : W - 1]
            dst = pt[:, :, 1:W]
        else:
            rhs = xp[:, h0 : h0 + rows, 1:W]
            dst = pt[:, :, 0 : W - 1]
        nc.tensor.matmul(dst, wbuf16[:, oc, t, :], rhs, start=(t == 0), stop=False)
    # fp8 DoubleRow pair: rhs [C, 2, rows*W] (host-shifted planes), out = full band
    nc.tensor.matmul(
        pt[:],
        wbuf8[:, oc],
        ypl[:, :, r0 * W : r0 * W + n16],
        start=False,
        stop=True,
        perf_mode=mybir.MatmulPerfMode.DoubleRow,
    )


def _build():
    nc = bacc.Bacc(None, target_bir_lowering=False)
    x = nc.dram_tensor("x", [BPC, C, HP, W], F16, kind="ExternalInput")
    y = nc.dram_tensor("y", [BPC, C, 2, HW], F8, kind="ExternalInput")
    w16 = nc.dram_tensor("w16", [2, C, 7, 128], F16, kind="ExternalInput")
    w8 = nc.dram_tensor("w8", [2, C, 2, 128], F8, kind="ExternalInput")
    out = nc.dram_tensor("out", [BPC, COUT, H, W], F16, kind="ExternalOutput")

    with tile.TileContext(nc) as tc:
        with (
            tc.tile_pool(name="wpool", bufs=1) as wpool,
            tc.tile_pool(name="xpool", bufs=2) as xpool,
            tc.tile_pool(name="ypool", bufs=2) as ypool,
            tc.tile_pool(name="opool", bufs=2) as opool,
            tc.tile_pool(name="pspool", bufs=8, space="PSUM") as pspool,
        ):
            warm = wpool.tile([C, 448], BF16)
            nc.vector.memset(warm[:], 0.0)

            wbuf16 = wpool.tile([C, 2, 7, 128], F16)
            wbuf8 = wpool.tile([C, 2, 2, 128], F8)
            xp0 = xpool.tile([C, HP, W], F16, tag="xp", name="xp0")
            ypl0 = ypool.tile([C, 2, HW], F8, tag="ypl", name="ypl0")
            # sync ring: oc=0 weights, then the image-0 bulk; scalar ring:
            # the band-0/1 head of image 0 + y planes in parallel. This exact
            # arrangement measured best; deviations starve the early bands
            # (each extra/reordered DMA shifts ~0.7us of serialized ring time
            # and a starved band makes the HAM clock re-throttle).
            nc.sync.dma_start(wbuf16[:, 0], w16[0])
            nc.sync.dma_start(wbuf8[:, 0], w8[0])
            nc.scalar.dma_start(xp0[:, 0:18, :], x[0, :, 0:18, :])
            nc.scalar.dma_start(ypl0[:, :, 0 : 16 * W], y[0, :, :, 0 : 16 * W])
            nc.sync.dma_start(xp0[:, 18:HP, :], x[0, :, 18:HP, :])
            nc.scalar.dma_start(ypl0[:, :, 16 * W : HW], y[0, :, :, 16 * W : HW])
            nc.sync.dma_start(wbuf16[:, 1], w16[1])
            nc.sync.dma_start(wbuf8[:, 1], w8[1])

            # PE warmup: keeps the HAM activity monitor busy from ~6.7us so the
            # clock is at 8/8 by the time the first data-dependent matmul runs.
            wpsum = pspool.tile([16, 448], F32, tag="pt", name="warm_psum")
            for i in range(NWARM):
                nc.tensor.matmul(wpsum[:], warm[:, :16], warm[:],
                                 start=(i == 0), stop=(i == NWARM - 1))

            for b in range(BPC):
                if b == 0:
                    xp, ypl = xp0, ypl0
                else:
                    xp = xpool.tile([C, HP, W], F16, tag="xp", name=f"xp{b}")
                    ypl = ypool.tile([C, 2, HW], F8, tag="ypl", name=f"ypl{b}")
                    nc.sync.dma_start(xp[:], x[b])
                    nc.scalar.dma_start(ypl[:], y[b])

                for oc in range(COUT // 128):
                    oimg = opool.tile([128, H, W], F16, tag="oimg", name=f"oimg{b}_{oc}")
                    last_img = b == BPC - 1 and oc == 1
                    for ht in range(HT):
                        if last_img and ht == HT - 1:
                            # final band split 6+2 so only a tiny copy+DMA
                            # trails the last matmul
                            for p0, rows in ((0, 6), (6, 2)):
                                r0 = ht * HROWS + p0
                                pt = pspool.tile([128, rows, W], F32, tag="pt",
                                                 name=f"pt{b}_{oc}_{ht}_{p0}")
                                _band(nc, pt, xp, ypl, wbuf16, wbuf8, oc, r0, rows)
                                nc.vector.tensor_copy(
                                    out=oimg[:, r0 : r0 + rows, :], in_=pt[:])
                                eng = nc.scalar if p0 else nc.sync
                                eng.dma_start(
                                    out[b, oc * 128 : (oc + 1) * 128, r0 : r0 + rows, :],
                                    oimg[:, r0 : r0 + rows, :])
                        else:
                            r0 = ht * HROWS
                            pt = pspool.tile([128, HROWS, W], F32, tag="pt",
                                             name=f"pt{b}_{oc}_{ht}")
                            _band(nc, pt, xp, ypl, wbuf16, wbuf8, oc, r0, HROWS)
                            nc.vector.tensor_copy(
                                out=oimg[:, r0 : r0 + HROWS, :], in_=pt[:])
                            # merge two bands per out-DMA (and the odd 7th
                            # alone) to halve ring occupancy
                            if ht % 2 == 1 or (ht == HT - 1 and not last_img):
                                d0 = r0 - HROWS if ht % 2 == 1 else r0
                                eng = nc.scalar if (b * 2 + oc * 7 + ht) % 2 else nc.sync
                                eng.dma_start(
                                    out[b, oc * 128 : (oc + 1) * 128, d0 : r0 + HROWS, :],
                                    oimg[:, d0 : r0 + HROWS, :])
    nc.compile()
    return nc


def _get_nc():
    global _cached_nc
    if _cached_nc is None:
        _cached_nc = _build()
    return _cached_nc


def _prep_inputs(x, w):
    x = np.ascontiguousarray(x, dtype=np.float32)
    # fp16 image, h-padded (rows 0 and 57 zero)
    x16 = np.zeros((B, C, HP, W), dtype=F16_NP)
    x16[:, :, 1 : H + 1, :] = x.astype(F16_NP)
    # fp8 shifted planes for the (0,-1)/(0,+1) pair, edge zeros baked in
    x8 = x.astype(F8_NP)
    y8 = np.zeros((B, C, 2, H, W), dtype=F8_NP)
    y8[:, :, 0, :, 1:] = x8[:, :, :, : W - 1]   # plane0[h,w] = x[h, w-1]
    y8[:, :, 1, :, : W - 1] = x8[:, :, :, 1:]   # plane1[h,w] = x[h, w+1]
    y8 = y8.reshape(B, C, 2, HW)

    wf = np.asarray(w, dtype=np.float32).reshape(C, 9, 2, 128) * np.float32(WSCALE)
    kk16 = [(dh + 1) * 3 + (dw + 1) for dh, dw in TAPS16]
    kk8 = [(dh + 1) * 3 + (dw + 1) for dh, dw in PAIR]
    # [2, C, ntap, 128]
    w16 = np.ascontiguousarray(wf[:, kk16].transpose(2, 0, 1, 3)).astype(F16_NP)
    w8 = np.ascontiguousarray(wf[:, kk8].transpose(2, 0, 1, 3)).astype(F8_NP)
    return x16, y8, w16, w8


def run(x, w, trace=False, **spmd_kwargs):
    nc = _get_nc()
    x16, y8, w16, w8 = _prep_inputs(x, w)
    in_maps = [
        {
            "x": x16[i * BPC : (i + 1) * BPC],
            "y": y8[i * BPC : (i + 1) * BPC],
            "w16": w16,
            "w8": w8,
        }
        for i in range(NCORES)
    ]
    res = bass_utils.run_bass_kernel_spmd(
        nc, in_maps, core_ids=list(range(NCORES)), trace=trace, **spmd_kwargs
    )
    full = np.concatenate([r["out"] for r in res.results], axis=0)
    full = full.astype(np.float32) * np.float32(1.0 / WSCALE)
    return full, res


def kernel(x, w):
    return run(x, w)[0]


# revision 15
# speedup vs baseline: 1.1724x; 1.1724x over previous
"""3x3 conv (im2col formulation) on TRN2, data-parallel over batch, 8 cores.

Full inputs: x [32, 128, 56, 56] f32, w [1152, 256] f32 (row = c*9 + kh*3 + kw).
Full output: [32, 256, 56, 56] f32.

Each core processes 4 images; no collectives. Per core, per (image, oc-half,
8-row band) the 9 conv taps accumulate w_tap.T @ x_shifted into a [128, 8, 56]
f32 PSUM bank:
  - 7 taps run in fp16 (host-cast, w pre-scaled by 64 = exponent shift):
    3 full-width (dw=0, N=448) + 4 width-55 (dh=+-1, dw=+-1) matmuls.
  - The 2 (dh=0, dw=+-1) taps are fused into ONE fp8e4 DoubleRow matmul
    (2 weights/PE cell, 2 MACs/cycle): host bakes the +-1 column shifts
    (with edge zeros) into a [C, 2, H*W] plane tensor, so the pair is a
    full-width N=448 matmul streaming 896 fp8 values. Band cost drops from
    3984 to ~3610 PE cycles (-9.4%). Measured end-to-end rel err ~1.95e-2
    (gate 2e-2; deterministic: host-side round-to-nearest quantization,
    fixed PSUM accumulation order).
  - Output staged as fp16 carrying 64x values (PSUM copy converts); host
    divides by 64 after gather (exact, exponent-only).
Lead-in: first data chunks are split across both HWDGE rings (w16/w8 halves
on sync, x/y heads on scalar) so the first real matmul starts ~10us; 8 bf16
warmup matmuls keep the PE HAM activity window busy from ~6.7us so the real
stream runs at 2.4GHz. Tail: the last band is computed as 6-row + 2-row PSUM
groups so the final trailing DMA is small and the big chunk's HBM receipt
overlaps the 2-row group's matmuls.
"""

import numpy as np
import ml_dtypes

import concourse.bass as bass  # noqa: F401  (registers AP types)
import concourse.mybir as mybir
import concourse.tile as tile
from concourse import bacc, bass_utils

B, C, H, W = 32, 128, 56, 56
COUT = 256
NCORES = 8
BPC = B // NCORES  # images per core
HP = H + 2
HW = H * W
F32 = mybir.dt.float32
BF16 = mybir.dt.bfloat16
F16 = mybir.dt.float16
F8 = mybir.dt.float8e4
F16_NP = np.float16
F8_NP = ml_dtypes.float8_e4m3

# fp16 taps: full-width dw=0 first (N=448), then the four corner taps (N=440).
TAPS16 = [(-1, 0), (0, 0), (1, 0), (-1, -1), (-1, 1), (1, -1), (1, 1)]
PAIR = [(0, -1), (0, 1)]  # fused fp8 DoubleRow pair
WSCALE = 64.0  # pure exponent shift; undone on host after gather
HROWS = 8
HT = H // HROWS  # bands per image
NWARM = 8

_cached_nc = None


def _band(nc, pt, xp, ypl, wbuf16, wbuf8, oc, r0, rows):
    """Emit the 8 matmuls of one PSUM band covering output rows r0..r0+rows."""
    n16 = rows * W
    for t, (dh, dw) in enumerate(TAPS16):
        h0 = r0 + dh + 1
        if dw == 0:
            rhs = xp[:, h0 : h0 + rows, :]
            dst = pt[:]
        elif dw == -1:
            rhs = xp[:, h0 : h0 + rows, 0 : W - 1]
            dst = pt[:, :, 1:W]
        else:
            rhs = xp[:, h0 : h0 + rows, 1:W]
            dst = pt[:, :, 0 : W - 1]
        nc.tensor.matmul(dst, wbuf16[:, oc, t, :], rhs, start=(t == 0), stop=False)
    # fp8 DoubleRow pair: rhs [C, 2, rows*W] (host-shifted planes), out = full band
    nc.tensor.matmul(
        pt[:],
        wbuf8[:, oc],
        ypl[:, :, r0 * W : r0 * W + n16],
        start=False,
        stop=True,
        perf_mode=mybir.MatmulPerfMode.DoubleRow,
    )


def _build():
    nc = bacc.Bacc(None, target_bir_lowering=False)
    x = nc.dram_tensor("x", [BPC, C, HP, W], F16, kind="ExternalInput")
    y = nc.dram_tensor("y", [BPC, C, 2, HW], F8, kind="ExternalInput")
    w16 = nc.dram_tensor("w16", [2, C, 7, 128], F16, kind="ExternalInput")
    w8 = nc.dram_tensor("w8", [2, C, 2, 128], F8, kind="ExternalInput")
    out = nc.dram_tensor("out", [BPC, COUT, H, W], F16, kind="ExternalOutput")

    with tile.TileContext(nc) as tc:
        with (
            tc.tile_pool(name="wpool", bufs=1) as wpool,
            tc.tile_pool(name="xpool", bufs=2) as xpool,
            tc.tile_pool(name="ypool", bufs=2) as ypool,
            tc.tile_pool(name="opool", bufs=2) as opool,
            tc.tile_pool(name="pspool", bufs=8, space="PSUM") as pspool,
        ):
            warm = wpool.tile([C, 448], BF16)
            nc.vector.memset(warm[:], 0.0)

            wbuf16 = wpool.tile([C, 2, 7, 128], F16)
            wbuf8 = wpool.tile([C, 2, 2, 128], F8)
            xp0 = xpool.tile([C, HP, W], F16, tag="xp", name="xp0")
            ypl0 = ypool.tile([C, 2, HW], F8, tag="ypl", name="ypl0")
            # sync ring: oc=0 weights, then the image-0 bulk; scalar ring:
            # the band-0/1 head of image 0 + y planes in parallel. This exact
            # arrangement measured best; deviations starve the early bands
            # (each extra/reordered DMA shifts ~0.7us of serialized ring time
            # and a starved band makes the HAM clock re-throttle).
            nc.sync.dma_start(wbuf16[:, 0], w16[0])
            nc.sync.dma_start(wbuf8[:, 0], w8[0])
            nc.scalar.dma_start(xp0[:, 0:18, :], x[0, :, 0:18, :])
            nc.scalar.dma_start(ypl0[:, :, 0 : 16 * W], y[0, :, :, 0 : 16 * W])
            nc.sync.dma_start(xp0[:, 18:HP, :], x[0, :, 18:HP, :])
            nc.scalar.dma_start(ypl0[:, :, 16 * W : HW], y[0, :, :, 16 * W : HW])
            nc.sync.dma_start(wbuf16[:, 1], w16[1])
            nc.sync.dma_start(wbuf8[:, 1], w8[1])

            # PE warmup: keeps the HAM activity monitor busy from ~6.7us so the
            # clock is at 8/8 by the time the first data-dependent matmul runs.
            wpsum = pspool.tile([16, 448], F32, tag="pt", name="warm_psum")
            for i in range(NWARM):
                nc.tensor.matmul(wpsum[:], warm[:, :16], warm[:],
                                 start=(i == 0), stop=(i == NWARM - 1))

            for b in range(BPC):
                if b == 0:
                    xp, ypl = xp0, ypl0
                else:
                    xp = xpool.tile([C, HP, W], F16, tag="xp", name=f"xp{b}")
                    ypl = ypool.tile([C, 2, HW], F8, tag="ypl", name=f"ypl{b}")
                    nc.sync.dma_start(xp[:], x[b])
                    nc.scalar.dma_start(ypl[:], y[b])

                for oc in range(COUT // 128):
                    oimg = opool.tile([128, H, W], F16, tag="oimg", name=f"oimg{b}_{oc}")
                    last_img = b == BPC - 1 and oc == 1
                    for ht in range(HT):
                        if last_img and ht == HT - 1:
                            # final band split 6+2 so only a tiny copy+DMA
                            # trails the last matmul
                            for p0, rows in ((0, 6), (6, 2)):
                                r0 = ht * HROWS + p0
                                pt = pspool.tile([128, rows, W], F32, tag="pt",
                                                 name=f"pt{b}_{oc}_{ht}_{p0}")
                                _band(nc, pt, xp, ypl, wbuf16, wbuf8, oc, r0, rows)
                                nc.vector.tensor_copy(
                                    out=oimg[:, r0 : r0 + rows, :], in_=pt[:])
                                eng = nc.scalar if p0 else nc.sync
                                eng.dma_start(
                                    out[b, oc * 128 : (oc + 1) * 128, r0 : r0 + rows, :],
                                    oimg[:, r0 : r0 + rows, :])
                        else:
                            r0 = ht * HROWS
                            pt = pspool.tile([128, HROWS, W], F32, tag="pt",
                                             name=f"pt{b}_{oc}_{ht}")
                            _band(nc, pt, xp, ypl, wbuf16, wbuf8, oc, r0, HROWS)
                            nc.vector.tensor_copy(
                                out=oimg[:, r0 : r0 + HROWS, :], in_=pt[:])
                            # one DMA per band; merging two bands per DMA
                            # measured 19us WORSE (delays buffer releases)
                            eng = nc.scalar if (b * 2 + oc * 7 + ht) % 2 else nc.sync
                            eng.dma_start(
                                out[b, oc * 128 : (oc + 1) * 128, r0 : r0 + HROWS, :],
                                oimg[:, r0 : r0 + HROWS, :])
    nc.compile()
    return nc


def _get_nc():
    global _cached_nc
    if _cached_nc is None:
        _cached_nc = _build()
    return _cached_nc


def _prep_inputs(x, w):
    x = np.ascontiguousarray(x, dtype=np.float32)
    # fp16 image, h-padded (rows 0 and 57 zero)
    x16 = np.zeros((B, C, HP, W), dtype=F16_NP)
    x16[:, :, 1 : H + 1, :] = x.astype(F16_NP)
    # fp8 shifted planes for the (0,-1)/(0,+1) pair, edge zeros baked in
    x8 = x.astype(F8_NP)
    y8 = np.zeros((B, C, 2, H, W), dtype=F8_NP)
    y8[:, :, 0, :, 1:] = x8[:, :, :, : W - 1]   # plane0[h,w] = x[h, w-1]
    y8[:, :, 1, :, : W - 1] = x8[:, :, :, 1:]   # plane1[h,w] = x[h, w+1]
    y8 = y8.reshape(B, C, 2, HW)

    wf = np.asarray(w, dtype=np.float32).reshape(C, 9, 2, 128) * np.float32(WSCALE)
    kk16 = [(dh + 1) * 3 + (dw + 1) for dh, dw in TAPS16]
    kk8 = [(dh + 1) * 3 + (dw + 1) for dh, dw in PAIR]
    # [2, C, ntap, 128]
    w16 = np.ascontiguousarray(wf[:, kk16].transpose(2, 0, 1, 3)).astype(F16_NP)
    w8 = np.ascontiguousarray(wf[:, kk8].transpose(2, 0, 1, 3)).astype(F8_NP)
    return x16, y8, w16, w8


def run(x, w, trace=False, **spmd_kwargs):
    nc = _get_nc()
    x16, y8, w16, w8 = _prep_inputs(x, w)
    in_maps = [
        {
            "x": x16[i * BPC : (i + 1) * BPC],
            "y": y8[i * BPC : (i + 1) * BPC],
            "w16": w16,
            "w8": w8,
        }
        for i in range(NCORES)
    ]
    res = bass_utils.run_bass_kernel_spmd(
        nc, in_maps, core_ids=list(range(NCORES)), trace=trace, **spmd_kwargs
    )
    full = np.concatenate([r["out"] for r in res.results], axis=0)
    full = full.astype(np.float32) * np.float32(1.0 / WSCALE)
    return full, res


def kernel(x, w):
    return run(x, w)[0]
